# revision 36
# baseline (speedup 1.0000x reference)
"""Trainium2 Bass kernel for nn_MessagePN2 (per-batch MHA with tiny head dim + MLP).

Per-core work (data-parallel over batch, 1 batch element per core):
  q/k/v projections, scores = q @ k^T / sqrt(3) per head, attn = softmax(scores),
  out = (attn @ v) @ Wo, y = relu(relu([out|data] @ W1 + b1) @ W2 + b2).
Outputs per core: attn [H, N, N] fp32 (128 MiB) and y [N, C] fp32.

Design (engine budget per core, target ~500us):
  - scores are computed TWICE on the PE (cheap: streaming-bound, with row/col
    32-tiling): layout A [n, m] for the softmax + HBM attn write, and layout B
    [m, n] to feed attn @ v (PE contracts the partition dim, so A@V needs m on
    partitions; transposing 128 MiB through any engine is slower than
    recomputing scores).
  - exp runs on ScalarE (the bottleneck: 2 passes over 33.5M elements).
    Pass A uses accum_out to get softmax row sums for free; VectorE then does
    the normalize multiply (fp32 tensor_scalar, 2x mode).
  - Path B: E_B = exp(scores_B) feeds AV matmuls with V augmented by a ones
    column, giving unnormalized O^T and the row sums; O is normalized with a
    small PE-transpose dance and projected through Wo; MLP runs on PE with
    per-partition biases.
"""

import numpy as np


def _import_concourse():
    try:
        import concourse.bass  # noqa: F401
    except ImportError:
        import sys

        for p in (
            "/root/.axon_site",
            "/root/.axon_site/_ro/trn_rl_repo",
            "/root/.axon_site/_ro/pypackages",
            "/opt/trn_rl_repo",
        ):
            if p not in sys.path:
                sys.path.append(p)
        import concourse.bass  # noqa: F401


B, N, C = 8, 2048, 64
H, DK = 8, 3
HDK = H * DK  # 24
HID = 1152
NCORES = 8
F32 = None  # set after import

_CACHE = {}


def _legalize_waits(nc):
    """Enforce <=1 sync wait per instruction (this walrus build's limit).

    Excess waits are relocated onto a preceding zero-wait instruction of the
    same engine (same-engine program order = block order). Safety: moving
    wait w (producer instruction S) onto anchor Y adds the constraint
    "Y after S"; a deadlock needs Y ~> S in the dependency graph, so we only
    pick anchors with Y not in ancestors(S), and afterwards re-verify the
    whole mutated graph is acyclic (Kahn).
    """
    from concourse import mybir

    insts = []
    for blk in nc.m.functions[0].blocks:
        insts.extend(list(blk.instructions))
    n = len(insts)

    # sem producer resolution: cumulative update_value per sem, in BLOCK order
    # (per-engine cumulative order is what matters; block order restricted to
    # one engine IS its program order, and sems are single-producer-proc here)
    def build_producers():
        events = {}  # sem -> list of (cum_value, idx)
        cum = {}
        for i, inst in enumerate(insts):
            si = inst.sync_info
            if not si:
                continue
            for u in si.on_update:
                c = cum.get(u.ant_name, 0) + (u.update_value or 1)
                cum[u.ant_name] = c
                events.setdefault(u.ant_name, []).append((c, i))
        return events

    events = build_producers()

    def pidx(w):
        # barrier EVSEMs use set/reset semantics our cumulative model can't
        # express; they only appear in Tile's tail barrier (1 wait each).
        if "barrier" in w.ant_name:
            return -1
        evs = events.get(w.ant_name)
        if not evs:
            return -1
        for c, i in evs:
            if c >= w.wait_value:
                return i
        return -1

    def edges_of(i):
        inst = insts[i]
        deps = []
        si = inst.sync_info
        if si:
            for w in si.on_wait:
                p = pidx(w)
                if p >= 0 and p != i:
                    deps.append(p)
        return deps

    def build_graph():
        # deps[i] = list of indices i depends on
        deps = [[] for _ in range(n)]
        last_on_engine = {}
        for i, inst in enumerate(insts):
            e = inst.engine
            if e in last_on_engine:
                deps[i].append(last_on_engine[e])
            last_on_engine[e] = i
            deps[i].extend(edges_of(i))
        return deps

    deps = build_graph()

    # ancestors via topo order (Kahn) with python-int bitsets
    def topo_and_ancestors(deps, want_anc=True):
        outs = [[] for _ in range(n)]
        indeg = [0] * n
        for i, ds in enumerate(deps):
            for d in ds:
                outs[d].append(i)
                indeg[i] += 1
        from collections import deque

        q = deque(i for i in range(n) if indeg[i] == 0)
        order = []
        anc = [0] * n if want_anc else None
        while q:
            i = q.popleft()
            order.append(i)
            if want_anc:
                a = anc[i]
                for d in deps[i]:
                    a |= anc[d] | (1 << d)
                anc[i] = a
            for j in outs[i]:
                indeg[j] -= 1
                if indeg[j] == 0:
                    q.append(j)
        return order, anc

    order, anc = topo_and_ancestors(deps)
    assert len(order) == n, "original dep graph has a cycle?!"

    NO_ANCHOR = ("InstEventSemaphore", "InstDrain", "InstISA")
    n_waits = [len(i.sync_info.on_wait) if i.sync_info else 0 for i in insts]

    moved = 0
    for i, inst in enumerate(insts):
        si = inst.sync_info
        if not si or len(si.on_wait) <= 1:
            continue
        best = {}
        for w in si.on_wait:
            assert w.wait_mode == "sem-ge-imm" and pidx(w) >= 0, (
                f"{inst.name}: unresolvable multi-wait {w.ant_name}"
            )
            k = w.ant_name
            if k not in best or w.wait_value > best[k].wait_value:
                best[k] = w
        waits = sorted(best.values(), key=lambda w: pidx(w))
        keep = waits[-1]
        for w in waits[:-1]:
            s = pidx(w)
            placed = False
            for j in range(i - 1, -1, -1):
                cand = insts[j]
                if cand.engine != inst.engine:
                    continue
                if type(cand).__name__ in NO_ANCHOR or n_waits[j] != 0:
                    continue
                if (anc[s] >> j) & 1:
                    continue  # S depends on cand -> would deadlock
                csi = cand.sync_info
                cand.sync_info = mybir.SyncInfo(
                    on_wait=[w], on_update=list(csi.on_update) if csi else []
                )
                n_waits[j] = 1
                placed = True
                moved += 1
                break
            if not placed:
                msg = [
                    f"no anchor for wait {w.ant_name}>={w.wait_value} of "
                    f"{inst.name} ({type(inst).__name__} {inst.engine}) at {i}, "
                    f"producer S at {s} ({insts[s].name})"
                ]
                cnt = 0
                for j in range(i - 1, -1, -1):
                    if insts[j].engine != inst.engine:
                        continue
                    msg.append(
                        f"  cand {j} {insts[j].name} {type(insts[j]).__name__} "
                        f"nw={n_waits[j]} inAncS={(anc[s] >> j) & 1}"
                    )
                    cnt += 1
                    if cnt >= 15:
                        break
                raise AssertionError("\n".join(msg))
        inst.sync_info = mybir.SyncInfo(
            on_wait=[keep], on_update=list(si.on_update)
        )
        n_waits[i] = 1

    # final validation: every inst <=1 wait; mutated graph acyclic
    for inst in insts:
        si = inst.sync_info
        assert not si or len(si.on_wait) <= 1, f"{inst.name} still multi-wait"
    deps2 = build_graph()
    order2, _ = topo_and_ancestors(deps2, want_anc=False)
    assert len(order2) == n, "wait relocation introduced a dependency cycle"
    return moved


def _build_nc():
    """Build the per-core Bass program (same SPMD program on all 8 cores)."""
    if "nc" in _CACHE:
        return _CACHE["nc"]

    _import_concourse()
    import concourse.bass as bass
    import concourse.tile as tile
    from concourse import masks, mybir
    from contextlib import ExitStack

    f32 = mybir.dt.float32
    Exp = mybir.ActivationFunctionType.Exp
    add = mybir.AluOpType.add
    mx = mybir.AluOpType.max
    ts = bass.ts

    inv_s3 = float(1.0 / np.sqrt(3.0))

    nc = bass.Bass()

    data_d = nc.dram_tensor("data", [N, C], f32, kind="ExternalInput")
    wqp_d = nc.dram_tensor("wqp", [C, 256], f32, kind="ExternalInput")
    wkp_d = nc.dram_tensor("wkp", [C, 256], f32, kind="ExternalInput")
    wv_d = nc.dram_tensor("wv", [C, 32], f32, kind="ExternalInput")
    wo_d = nc.dram_tensor("wo", [128, 128], f32, kind="ExternalInput")
    w1_d = nc.dram_tensor("w1", [2 * C, HID], f32, kind="ExternalInput")
    b1_d = nc.dram_tensor("b1s", [128, 9], f32, kind="ExternalInput")
    w2_d = nc.dram_tensor("w2s", [128, 9 * C], f32, kind="ExternalInput")
    b2_d = nc.dram_tensor("b2s", [C, 1], f32, kind="ExternalInput")

    attn_d = nc.dram_tensor("attn", [H, N, N], f32, kind="ExternalOutput")
    y_d = nc.dram_tensor("y", [N, C], f32, kind="ExternalOutput")

    with tile.TileContext(nc) as tc, ExitStack() as ctx:
        const = ctx.enter_context(tc.tile_pool(name="const", bufs=1))
        p_load = ctx.enter_context(tc.tile_pool(name="load", bufs=3))
        p_attn = ctx.enter_context(tc.tile_pool(name="attn", bufs=4))
        p_eb = ctx.enter_context(tc.tile_pool(name="eb", bufs=3))
        p_ht = ctx.enter_context(tc.tile_pool(name="ht", bufs=10))
        p_small = ctx.enter_context(tc.tile_pool(name="small", bufs=8))
        # PSUM: 8 banks total. scores: 2 x [128,1024] = 4 banks,
        # obig: 2 x [128,1024] = 4 banks.
        p_sc = ctx.enter_context(tc.tile_pool(name="psc", bufs=2, space="PSUM"))
        p_ob = ctx.enter_context(tc.tile_pool(name="pob", bufs=2, space="PSUM"))

        # ---- constants ----
        ident = const.tile([128, 128], f32)
        masks.make_identity(nc, ident[:])

        wqp = const.tile([C, 256], f32)
        nc.sync.dma_start(wqp[:], wqp_d[:])
        wkp = const.tile([C, 256], f32)
        nc.sync.dma_start(wkp[:], wkp_d[:])
        wv = const.tile([C, 32], f32)
        nc.sync.dma_start(wv[:], wv_d[:])
        wo = const.tile([128, 128], f32)
        nc.sync.dma_start(wo[:], wo_d[:])
        w1 = const.tile([2 * C, HID], f32)
        nc.sync.dma_start(w1[:], w1_d[:])
        b1s = const.tile([128, 9], f32)
        nc.sync.dma_start(b1s[:], b1_d[:])
        w2s = const.tile([128, 9 * C], f32)
        nc.sync.dma_start(w2s[:], w2_d[:])
        b2s = const.tile([C, 1], f32)
        nc.sync.dma_start(b2s[:], b2_d[:])

        from concourse.tile import add_dep_helper

        _mark = [None]

        def mark(bi):
            _mark[0] = bi.ins if hasattr(bi, "ins") else bi
            return bi

        def anchor(*engines):
            # nops used as wait-relocation anchors by _legalize_waits; the
            # nosync dep pins them near the current site in each engine's
            # stream (zero-dep nops otherwise all hoist to the stream front)
            for e in engines:
                bi = getattr(nc, e).nop(hint="anch", nofuse=True)
                if _mark[0] is not None:
                    add_dep_helper(
                        bi.ins if hasattr(bi, "ins") else bi,
                        _mark[0],
                        sync=False,
                        reason="anchor-order",
                    )

        # ---- data^T ([C, N] at partitions 0:64) and xT rows 64:128 ----
        xT = const.tile([128, N], f32)  # rows 0:64 = out_c^T (later), 64:128 = data^T
        dT = const.tile([C, N], f32)
        for mt in range(16):
            anchor("tensor", "tensor", "vector", "vector", "sync", "sync")
            dtile = p_load.tile([128, C], f32, tag="dload")
            mark(nc.sync.dma_start(dtile[:], data_d[ts(mt, 128), :]))
            tp = p_ob.tile([C, 128], f32, tag="obig")
            nc.tensor.transpose(tp[:], dtile[:], ident[:])
            nc.vector.tensor_copy(dT[:, ts(mt, 128)], tp[:])
            nc.vector.tensor_copy(xT[C : 2 * C, ts(mt, 128)], tp[:])

        # ---- projections ----
        # QT_rep[r]/KT_rep[r]: [128, N]; partitions 32j+0..2 hold head (4r+j)'s
        # q^T/k^T rows (32j+3..31 are zero via host padding of wqp/wkp).
        QT = [const.tile([128, N], f32, name=f"qt{r}") for r in range(2)]
        KT = [const.tile([128, N], f32, name=f"kt{r}") for r in range(2)]
        for r in range(2):
            for dst, wsrc in ((QT[r], wqp), (KT[r], wkp)):
                for half in range(2):
                    anchor("tensor", "tensor", "vector", "vector")
                    pp = p_sc.tile([128, 1024], f32, tag="scores")
                    for c2 in range(2):
                        mark(nc.tensor.matmul(
                            pp[:, ts(c2, 512)],
                            lhsT=wsrc[:, ts(r, 128)],
                            rhs=dT[:, half * 1024 + c2 * 512 : half * 1024 + (c2 + 1) * 512],
                            start=True,
                            stop=True,
                        ))
                    nc.vector.tensor_copy(dst[:, ts(half, 1024)], pp[:])

        # V_sb: [128, 512]; chunk mt cols [32mt : 32mt+32], col 32mt+4h+d =
        # V[mt*128+p, 3h+d] for d<3, col 32mt+4h+3 = 1.0 (ones for row sums).
        V_sb = const.tile([128, 512], f32)
        for mt in range(16):
            anchor("tensor", "tensor", "vector")
            vp = p_ob.tile([128, 32], f32, tag="obig")
            mark(nc.tensor.matmul(
                vp[:], lhsT=dT[:, ts(mt, 128)], rhs=wv[:], start=True, stop=True
            ))
            nc.vector.tensor_copy(V_sb[:, ts(mt, 32)], vp[:])
        nc.vector.memset(V_sb[:, 3:512:4], 1.0)

        # O_aug[r] [128, 2048]: rows 32j+d = (attn_unnorm @ V_{h=4r+j})^T for d<3,
        # row 32j+3 = softmax row sums; other rows junk (never read: wo rows 0).
        O_aug = [const.tile([128, N], f32, name=f"oaug{r}") for r in range(2)]
        O_norm = [const.tile([128, N], f32, name=f"onorm{r}") for r in range(2)]

        # ---- main interleaved loops ----
        # A-jobs: (h, nt) -> scores_A [n,m] tile, exp+accum, normalize, DMA out.
        a_jobs = [(h, nt) for nt in range(16) for h in range(8)]
        a_idx = 0

        def emit_a_job(h, nt):
            r, j = divmod(h, 4)
            anchor("vector", "sync")
            attn_t = p_attn.tile([128, N], f32, tag="attn")
            s2 = p_small.tile([128, 4], f32, tag="s2")
            for half in range(2):
                anchor("tensor", "tensor", "scalar", "scalar", "vector", "vector")
                pa = p_sc.tile([128, 1024], f32, tag="scores")
                for c2 in range(2):
                    mark(nc.tensor.matmul(
                        pa[:, ts(c2, 512)],
                        lhsT=QT[r][32 * j : 32 * j + 3, ts(nt, 128)],
                        rhs=KT[r][32 * j : 32 * j + 3,
                                  half * 1024 + c2 * 512 : half * 1024 + (c2 + 1) * 512],
                        start=True,
                        stop=True,
                        tile_position=(32 * j, 0),
                    ))
                nc.scalar.activation(
                    attn_t[:, ts(half, 1024)],
                    pa[:],
                    Exp,
                    scale=inv_s3,
                    accum_out=s2[:, half : half + 1],
                )
            nc.vector.tensor_add(s2[:, 2:3], s2[:, 0:1], s2[:, 1:2])
            nc.vector.reciprocal(s2[:, 3:4], s2[:, 2:3])
            for half in range(2):
                nc.vector.tensor_scalar_mul(
                    attn_t[:, ts(half, 1024)], attn_t[:, ts(half, 1024)], s2[:, 3:4]
                )
            nc.sync.dma_start(attn_d[h, ts(nt, 128), :], attn_t[:])

        # B-path: per (r, nhalf): accumulate O over all m tiles in one PSUM tile.
        for r in range(2):
            for nhalf in range(2):
                anchor("vector")
                o_t = p_ob.tile([128, 1024], f32, tag="obig")
                nc.vector.memset(o_t[:], 0.0)
                for mt in range(16):
                    for j in range(4):
                        h = 4 * r + j
                        anchor("tensor", "tensor", "scalar", "scalar")
                        pb = p_sc.tile([128, 1024], f32, tag="scores")
                        for c2 in range(2):
                            mark(nc.tensor.matmul(
                                pb[:, ts(c2, 512)],
                                lhsT=KT[r][32 * j : 32 * j + 3, ts(mt, 128)],
                                rhs=QT[r][32 * j : 32 * j + 3,
                                          nhalf * 1024 + c2 * 512 : nhalf * 1024 + (c2 + 1) * 512],
                                start=True,
                                stop=True,
                                tile_position=(32 * j, 0),
                            ))
                        eb = p_eb.tile([128, 1024], f32, tag="eb")
                        nc.scalar.activation(eb[:], pb[:], Exp, scale=inv_s3)
                        for c2 in range(2):
                            nc.tensor.matmul(
                                o_t[32 * j : 32 * j + 4, ts(c2, 512)],
                                lhsT=V_sb[:, 32 * mt + 4 * h : 32 * mt + 4 * h + 4],
                                rhs=eb[:, ts(c2, 512)],
                                start=(mt == 0),
                                stop=(mt == 15),
                                tile_position=(0, 32 * j),
                            )
                    # interleave 2 A-jobs per (mt) so ScalarE/DMA stay evenly fed
                    for _ in range(2):
                        if a_idx < len(a_jobs):
                            emit_a_job(*a_jobs[a_idx])
                            a_idx += 1
                anchor("vector")
                mark(nc.vector.tensor_copy(O_aug[r][:, ts(nhalf, 1024)], o_t[:]))
        while a_idx < len(a_jobs):
            emit_a_job(*a_jobs[a_idx])
            a_idx += 1

        # ---- normalize O (transpose dance) ----
        # O_aug[r] [128, n] -> natural [n, 128] via PE transpose, divide cols
        # 32j+d by the sums col 32j+3, transpose back -> O_norm[r].
        for r in range(2):
            for nt in range(16):
                anchor("tensor", "tensor", "tensor", "vector", "vector", "vector")
                tpo = p_ob.tile([128, 128], f32, tag="obig")
                mark(nc.tensor.transpose(tpo[:], O_aug[r][:, ts(nt, 128)], ident[:]))
                r4 = p_small.tile([128, 4], f32, tag="r4")
                nc.vector.reciprocal(r4[:], tpo[:, 3:128:32])
                on_sb = p_small.tile([128, 128], f32, tag="onat")
                nc.vector.tensor_copy(on_sb[:], tpo[:])
                for d in range(3):
                    nc.vector.tensor_mul(on_sb[:, d:128:32], tpo[:, d:128:32], r4[:])
                tpo2 = p_ob.tile([128, 128], f32, tag="obig")
                nc.tensor.transpose(tpo2[:], on_sb[:], ident[:])
                nc.vector.tensor_copy(O_norm[r][:, ts(nt, 128)], tpo2[:])

        # ---- Wo projection -> xT rows 0:64 (out_c^T) ----
        # wo (host-packed [128, 128]): wo[32j+d, 64r+c] = Wo[3(4r+j)+d, c] for
        # d<3, zero rows elsewhere (kills the sums rows and junk rows).
        for nch in range(4):
            anchor("tensor", "vector")
            ocp = p_ob.tile([C, 512], f32, tag="obig")
            for r in range(2):
                mark(nc.tensor.matmul(
                    ocp[:],
                    lhsT=wo[:, ts(r, C)],
                    rhs=O_norm[r][:, ts(nch, 512)],
                    start=(r == 0),
                    stop=(r == 1),
                ))
            nc.vector.tensor_copy(xT[0:C, ts(nch, 512)], ocp[:])

        # ---- MLP ----
        for nch in range(4):
            hts = []
            for hc in range(9):
                anchor("tensor", "tensor", "vector", "vector")
                hp = p_ob.tile([128, 512], f32, tag="obig")
                mark(nc.tensor.matmul(
                    hp[:],
                    lhsT=w1[:, ts(hc, 128)],
                    rhs=xT[:, ts(nch, 512)],
                    start=True,
                    stop=True,
                ))
                ht = p_ht.tile([128, 512], f32, tag="ht")
                nc.vector.tensor_scalar(
                    ht[:], hp[:], b1s[:, hc : hc + 1], 0.0, op0=add, op1=mx
                )
                hts.append(ht)
            anchor("tensor", "vector")
            ytp = p_ob.tile([C, 512], f32, tag="obig")
            for hc in range(9):
                nc.tensor.matmul(
                    ytp[:],
                    lhsT=w2s[:, ts(hc, C)],
                    rhs=hts[hc][:],
                    start=(hc == 0),
                    stop=(hc == 8),
                )
            yts = p_small.tile([C, 512], f32, tag="yts")
            nc.vector.tensor_scalar(
                yts[:], ytp[:], b2s[:], 0.0, op0=add, op1=mx
            )
            for k in range(4):
                anchor("tensor", "tensor", "vector", "vector", "sync")
                ytr = p_ob.tile([128, C], f32, tag="obig")
                mark(nc.tensor.transpose(ytr[:], yts[:, ts(k, 128)], ident[0:C, 0:C]))
                ysb = p_small.tile([128, C], f32, tag="ysb")
                nc.vector.tensor_copy(ysb[:], ytr[:])
                nc.sync.dma_start(y_d[ts(nch * 4 + k, 128), :], ysb[:])

        # end-of-kernel anchors: absorb the tail drain's multi-proc waits
        for _ in range(14):
            anchor("sync")
        anchor("tensor", "vector", "scalar", "tensor", "vector", "scalar")

    moved = _legalize_waits(nc)
    _CACHE["nc"] = nc
    _CACHE["moved_waits"] = moved
    return nc


def _host_prep(inputs):
    """Shard + host-side weight packing. Returns per-core input maps."""
    data = np.ascontiguousarray(np.asarray(inputs["data"], dtype=np.float32))
    Wq = np.asarray(inputs["Wq"], dtype=np.float32)
    Wk = np.asarray(inputs["Wk"], dtype=np.float32)
    Wv = np.asarray(inputs["Wv"], dtype=np.float32)
    Wo = np.asarray(inputs["Wo"], dtype=np.float32)
    W1 = np.ascontiguousarray(np.asarray(inputs["W1"], dtype=np.float32))
    b1 = np.asarray(inputs["b1"], dtype=np.float32)
    W2 = np.asarray(inputs["W2"], dtype=np.float32)
    b2 = np.asarray(inputs["b2"], dtype=np.float32)

    # wqp/wkp: [64, 256]; round r block [:, 128r:128r+128]; head (4r+j) cols
    # 32j+0..2, rest zero.
    wqp = np.zeros((C, 256), np.float32)
    wkp = np.zeros((C, 256), np.float32)
    for h in range(8):
        r, j = divmod(h, 4)
        wqp[:, 128 * r + 32 * j : 128 * r + 32 * j + 3] = Wq[:, 3 * h : 3 * h + 3]
        wkp[:, 128 * r + 32 * j : 128 * r + 32 * j + 3] = Wk[:, 3 * h : 3 * h + 3]
    # wv_aug: [64, 32], col 4h+d = Wv[:, 3h+d] for d<3, col 4h+3 = 0 (ones set on device)
    wv_aug = np.zeros((C, 32), np.float32)
    wo_pack = np.zeros((128, 128), np.float32)
    for h in range(8):
        r, j = divmod(h, 4)
        wv_aug[:, 4 * h : 4 * h + 3] = Wv[:, 3 * h : 3 * h + 3]
        wo_pack[32 * j : 32 * j + 3, C * r : C * (r + 1)] = Wo[3 * h : 3 * h + 3, :]
    b1s = np.ascontiguousarray(b1.reshape(9, 128).T)  # [128, 9]
    w2s = np.ascontiguousarray(
        W2.reshape(9, 128, C).transpose(1, 0, 2).reshape(128, 9 * C)
    )
    b2s = np.ascontiguousarray(b2.reshape(C, 1))

    shared = {
        "wqp": wqp,
        "wkp": wkp,
        "wv": wv_aug,
        "wo": wo_pack,
        "w1": W1,
        "b1s": b1s,
        "w2s": w2s,
        "b2s": b2s,
    }
    return [
        {"data": np.ascontiguousarray(data[b]), **shared} for b in range(NCORES)
    ]


def kernel(**inputs):
    _import_concourse()
    from concourse.bass_utils import run_bass_kernel_spmd

    nc = _build_nc()
    in_maps = _host_prep(inputs)
    res = run_bass_kernel_spmd(nc, in_maps, list(range(NCORES)))
    y = np.stack([np.asarray(res.results[b]["y"]) for b in range(NCORES)])
    attn = np.stack([np.asarray(res.results[b]["attn"]) for b in range(NCORES)])
    return (y, attn)
